# revision 55
# baseline (speedup 1.0000x reference)
"""Trainium2 Bass kernel: fused attention block (QKV proj -> MHA -> out proj).

Reference (per batch item b, NUM_HEADS=12, Dh=64):
    qkv = x @ W_qkv; q,k,v per head
    attn = softmax(q @ k^T / 8) @ v
    out  = concat_heads(attn) @ W_proj + b_proj

Sharding: data-parallel over batch across 8 NeuronCores (128 batch items
per core), weights replicated. One SPMD Bass program, per-core inputs.

Design (bf16 + fp8 k-chains, feature-major I/O, half-array head pairing,
fully software-pipelined):
  - Host pre-transposes x to feature-major [C, TOK] bf16 (and an fp8
    copy for the k-projection) and transposes the feature-major bf16
    output back; device never transposes.
  - Groups of G=8 batches (T=392 tokens). Per group:
    B: q co-tiles [128, T] = Wq_slice.T @ xT in bf16; k co-tiles via
       fp8 DoubleRow matmuls (2 c-chunks contracted per instruction at
       2x rate) from the fp8 copies of x and W_k. Scores tolerate the
       one-sided fp8 noise (k fp8, q bf16: rel err 1.4e-2 < 2e-2).
    C: v token-major in 4 tiles of 98 tokens, scattered by SBUF->SBUF
       DMA into per-(head-pair j, batch b) blocks vbd2 [128, 64]:
       rows 0:49 = even-head key positions, 64:113 = odd-head.
    D: per j: 8 batches x 2 half-array matmuls (even head in array
       quadrants (0:64)x(0:64), odd head at tile_position=(64,64))
       for scores and attn@V; single-op exp on ACT; row sums via ones
       matmul; reciprocal on DVE; the per-token reciprocal is
       broadcast across partitions on the GPSIMD engine
       (partition_broadcast) instead of a tensor-engine broadcast
       matmul, keeping the PE free for real GEMM work.
    E: out co-tiles [128, T] = Wproj_slice.T @ unT + per-partition bias.
  - Steady state runs D in head-pair double-steps and interleaves E of
    the previous group plus B and C of the next group into them, so the
    tensor engine sees one dense instruction stream and the HAM clock
    gate stays at full rate.
"""
import sys

sys.path.insert(0, "/opt/trn_rl_repo")

import numpy as np
import ml_dtypes

NUM_CORES = 8
B_CORE = 128          # batch items per core
SEQ = 49              # tokens per batch item
C = 768               # channels
H = 12                # heads
G = 8                 # batch items per group
T = SEQ * G           # 392 tokens per group
TOK = B_CORE * SEQ    # 6272 tokens per core
N_GROUPS = B_CORE // G

BF = ml_dtypes.bfloat16
F8 = ml_dtypes.float8_e4m3fn

_CACHE = {}


def _consts():
    onesbd = np.zeros((128, 2), dtype=BF)
    onesbd[0:49, 0] = 1.0
    onesbd[64:113, 1] = 1.0
    return {"onesbd": onesbd}


def _build():
    import concourse.bacc as bacc
    import concourse.mybir as mybir
    import concourse.tile as tile

    F32 = mybir.dt.float32
    BF16 = mybir.dt.bfloat16
    FP8 = mybir.dt.float8e4
    EXP = mybir.ActivationFunctionType.Exp
    DR = mybir.MatmulPerfMode.DoubleRow

    nc = bacc.Bacc("TRN2", target_bir_lowering=False)

    # x/x8/out are host-tiled to partition-major [128, 6*TOK] so every
    # DMA's source run structure matches the SBUF destination exactly
    # (collapses Sync-engine descriptor generation)
    d_x = nc.declare_dram_parameter("x", [128, 6 * TOK], BF16,
                                    isOutput=False)
    d_x8 = nc.declare_dram_parameter("x8", [128, 6 * TOK], FP8,
                                     isOutput=False)
    # bf16 weights: q columns (0:C) and v columns (C:2C of this tensor)
    d_wqv = nc.declare_dram_parameter("wqv", [C, 2 * C], BF16, isOutput=False)
    d_wk8 = nc.declare_dram_parameter("wk8", [C, C], FP8, isOutput=False)
    d_wproj = nc.declare_dram_parameter("wproj", [C, C], BF16, isOutput=False)
    d_bias = nc.declare_dram_parameter("bias", [128, 6], F32, isOutput=False)
    d_onesbd = nc.declare_dram_parameter("onesbd", [128, 2], BF16,
                                         isOutput=False)
    d_out = nc.declare_dram_parameter("out", [128, 6 * TOK], BF16,
                                      isOutput=True)

    # DRAM views with the 6x128 channel-tile structure exposed, so one DMA
    # moves all 6 channel tiles of a slice
    x6 = d_x.rearrange("p (c t) -> p c t", t=TOK)
    x86 = d_x8.rearrange("p (c t) -> p c t", t=TOK)
    out6 = d_out.rearrange("p (c t) -> p c t", t=TOK)
    wqv6 = d_wqv.rearrange("(c p) n -> p c n", p=128)
    wk86 = d_wk8.rearrange("(c p) n -> p c n", p=128)
    wproj6 = d_wproj.rearrange("(c p) n -> p c n", p=128)

    with tile.TileContext(nc) as tc, \
         nc.allow_low_precision(reason="bf16/fp8 matmuls within 2e-2 tol"):
        with tc.tile_pool(name="wres", bufs=1) as wres, \
             tc.tile_pool(name="qk", bufs=2) as p_qk, \
             tc.tile_pool(name="scr", bufs=2) as p_scr, \
             tc.tile_pool(name="vbd2", bufs=3) as p_vbd2, \
             tc.tile_pool(name="rr", bufs=2) as p_rr, \
             tc.tile_pool(name="rb", bufs=2) as p_rb, \
             tc.tile_pool(name="unT", bufs=2) as p_unT, \
             tc.tile_pool(name="osb", bufs=2) as p_osb, \
             tc.tile_pool(name="psA", bufs=2, space="PSUM") as psA, \
             tc.tile_pool(name="psS", bufs=2, space="PSUM") as psS, \
             tc.tile_pool(name="psO", bufs=2, space="PSUM") as psO, \
             tc.tile_pool(name="psV", bufs=2, space="PSUM") as psV:

            # ---- resident weights / constants ----
            # q weight columns first: the first B-stage matmuls need
            # only these plus the group-0 x tiles
            w_qv = wres.tile([128, 6 * 2 * C], BF16, tag="wqv", name="wqv")
            wq6 = w_qv.rearrange("p (c n) -> p c n", n=2 * C)
            # critical-path-first startup: the first B chain needs only
            # wq cols 0:128 plus x chunk ci=0, so issue those ahead of
            # the remaining ~2MB of weights
            nc.sync.dma_start(wq6[:, :, 0:128], wqv6[:, :, 0:128])

            S = [dict() for _ in range(N_GROUPS)]

            # x resident feature-major for the whole core: B slices it per
            # group, C slices it in clean 128-token tiles across group
            # boundaries (no runt tiles, full partition utilization).
            # x8 is the fp8 copy feeding the DoubleRow k-chains.
            xTall = wres.tile([128, 6 * TOK], BF16, tag="xTall", name="xTall")
            xv = xTall.rearrange("p (c t) -> p c t", t=TOK)
            x8all = wres.tile([128, 6 * TOK], FP8, tag="x8all", name="x8all")
            x8v = x8all.rearrange("p (c t) -> p c t", t=TOK)

            def load_xT(g, by_ci=False, x8=True):
                if by_ci:
                    # per-channel-chunk loads: the first B chain's ci-th
                    # matmul can start as soon as chunk ci has landed
                    for ci in range(6):
                        nc.sync.dma_start(
                            xv[:, ci, g * T:(g + 1) * T],
                            x6[:, ci, g * T:(g + 1) * T])
                else:
                    nc.sync.dma_start(
                        xv[:, :, g * T:(g + 1) * T],
                        x6[:, :, g * T:(g + 1) * T])
                if x8:
                    nc.sync.dma_start(x8v[:, :, g * T:(g + 1) * T],
                                      x86[:, :, g * T:(g + 1) * T])

            load_xT(0, by_ci=True, x8=False)
            # rest of the q weights, then the fp8 copies (k chains run
            # after all six q chains in the bootstrap)
            for o in range(128, C, 128):
                nc.sync.dma_start(wq6[:, :, o:o + 128],
                                  wqv6[:, :, o:o + 128])
            nc.sync.dma_start(x8v[:, :, 0:T], x86[:, :, 0:T])
            # fp8 k weights (needed by the k chains of group 0)
            wk8 = wres.tile([128, 6 * C], FP8, tag="wk8", name="wk8")
            wk6 = wk8.rearrange("p (c n) -> p c n", n=C)
            for o in range(0, C, 384):
                nc.sync.dma_start(wk6[:, :, o:o + 384],
                                  wk86[:, :, o:o + 384])
            # v weight columns (C stage)
            for o in range(C, 2 * C, 384):
                nc.sync.dma_start(wq6[:, :, o:o + 384],
                                  wqv6[:, :, o:o + 384])
            w_proj = wres.tile([128, 6 * C], BF16, tag="wproj", name="wproj")
            wp6 = w_proj.rearrange("p (c n) -> p c n", n=C)
            for o in range(0, C, 384):
                nc.sync.dma_start(wp6[:, :, o:o + 384],
                                  wproj6[:, :, o:o + 384])
            onesbd = wres.tile([128, 2], BF16, tag="onesbd")
            nc.sync.dma_start(onesbd[:], d_onesbd[:])
            bias_sb = wres.tile([128, 6], F32, tag="bias_sb")
            nc.sync.dma_start(bias_sb[:], d_bias[:])
            # exp tiles: dead bands (rows 49:64, 113:128) must stay zero.
            # 4 buffers: the tail pipeline is 2 steps deep, and each step
            # holds a pair (even j, odd j), so a tile written at step s is
            # still read at s+1 and can only be reused at s+2.
            eTs = []
            for nm in ("eTa0", "eTb0", "eTa1", "eTb1"):
                t = wres.tile([128, T], BF16, tag=nm, name=nm)
                nc.vector.memset(t[:], 0.0)
                eTs.append(t)
            # normalized exp tiles (only rows 0:49 / 64:113 ever read)
            eTns = []
            for nm in ("eTnA", "eTnB"):
                t = wres.tile([128, T], BF16, tag=nm, name=nm)
                eTns.append(t)
            # persistent double-buffered reciprocal staging tiles: the
            # 32-lane shuffle reads all 32 partitions, so they must be
            # fully initialized once (rows 2:32 stay at the memset value)
            rrbs, rros = [], []
            for nm in ("rrbA", "rrbB"):
                t = wres.tile([32, T], BF16, tag=nm, name=nm)
                nc.vector.memset(t[:], 0.0)
                rrbs.append(t)
            for nm in ("rroA", "rroB"):
                t = wres.tile([32, T], BF16, tag=nm, name=nm)
                nc.vector.memset(t[:], 0.0)
                rros.append(t)

            # ---- emission helpers ----
            def emit_B_chain(g, jc):
                st = S[g]
                if jc == 0:
                    st["q"], st["k"] = [None] * 6, [None] * 6
                pq = psA.tile([128, T], F32, tag="psA", name="psA")
                if jc < 6:
                    for ci in range(6):
                        nc.tensor.matmul(
                            pq[:], wq6[:, ci, 128 * jc:128 * (jc + 1)],
                            xv[:, ci, g * T:(g + 1) * T],
                            start=(ci == 0), stop=(ci == 5))
                else:
                    # fp8 DoubleRow: contract two c-chunks per matmul at
                    # 2x rate. 196-col halves keep the moving AP free dim
                    # at 392 <= 512 (the fast path). Both halves form ONE
                    # accumulation group: only the very first matmul sets
                    # start, so the bank's pending-zero covers the second
                    # half's columns instead of being re-marked (which
                    # would wipe the first half's results).
                    jk = jc - 6
                    for hh in range(2):
                        ts0 = g * T + 196 * hh
                        for c3 in range(3):
                            nc.tensor.matmul(
                                pq[:, 196 * hh:196 * (hh + 1)],
                                wk6[:, 2 * c3:2 * c3 + 2,
                                    128 * jk:128 * (jk + 1)],
                                x8v[:, 2 * c3:2 * c3 + 2, ts0:ts0 + 196],
                                start=(hh == 0 and c3 == 0),
                                stop=(hh == 1 and c3 == 2),
                                perf_mode=DR)
                nm = f"q{jc}" if jc < 6 else f"k{jc - 6}"
                t = p_qk.tile([128, T], BF16, tag=nm, name=nm)
                # all chain copies on ACT: GPSIMD cannot access PSUM
                # (BIR rule), and a DVE copy queues behind the deep
                # recip/shuffle/normalize chain, gating psA rotation
                nc.scalar.copy(t[:], pq[:])
                if jc < 6:
                    st["q"][jc] = t
                else:
                    st["k"][jc - 6] = t

            c_next = [0]

            def emit_C_unit():
                t4 = c_next[0]
                c_next[0] += 1
                tok0 = 128 * t4
                # scr layout [p, (parity, j, c)]: even-head features in
                # cols 0:384, odd in 384:768, so each scatter side is one
                # contiguous 384-element run per partition (cheap DMA
                # descriptor generation on the Sync engine)
                # host permuted the W_v columns to (parity, pair, c) order,
                # so half 0 of the GEMM output is all even-head features
                # and half 1 all odd-head: the copy is contiguous and each
                # scatter side is one 384-element run per partition
                scr = p_scr.tile([128, C], BF16, tag="scr", name="scr")
                for half in range(2):
                    pv = psS.tile([128, 384], F32, tag="psS", name="psS")
                    for ci in range(6):
                        nc.tensor.matmul(
                            pv[:], xv[:, ci, tok0:tok0 + 128],
                            wq6[:, ci, 768 + 384 * half:
                                768 + 384 * (half + 1)],
                            start=(ci == 0), stop=(ci == 5))
                    nc.vector.tensor_copy(
                        scr[:, 384 * half:384 * (half + 1)], pv[:])
                for b in range(tok0 // SEQ, min(B_CORE, (tok0 + 127) // SEQ + 1)):
                    lo = max(SEQ * b, tok0)
                    hi = min(SEQ * (b + 1), tok0 + 128)
                    if lo >= hi:
                        continue
                    gb, bb = b // G, b % G
                    if "v4" not in S[gb]:
                        vbd2 = p_vbd2.tile([128, G * 6 * 64], BF16,
                                           tag="vbd2", name="vbd2")
                        # [p, b, j, c]: per-(batch) 384-element contiguous
                        S[gb]["v4"] = vbd2.rearrange(
                            "p (b j c) -> p b j c", b=G, c=64)
                        S[gb]["v4f"] = vbd2.rearrange(
                            "p (b n) -> p b n", b=G)
                    v4f = S[gb]["v4f"]
                    sl, sh = lo - SEQ * b, hi - SEQ * b
                    nc.sync.dma_start(v4f[sl:sh, bb, :],
                                      scr[lo - tok0:hi - tok0, 0:384])
                    nc.sync.dma_start(v4f[64 + sl:64 + sh, bb, :],
                                      scr[lo - tok0:hi - tok0, 384:768])

            def ensure_C(tok_thresh):
                while c_next[0] < TOK // 128 and 128 * c_next[0] < tok_thresh:
                    emit_C_unit()

            def d_head(g, j):
                st = S[g]
                if j == 0:
                    st["unT"] = [p_unT.tile([128, T], BF16, tag=f"unT{ci}",
                                            name=f"unT{ci}")
                                 for ci in range(6)]
                    st["stash"] = {}
                q, k = st["q"], st["k"]
                eT = eTs[2 * (step_idx[0] % 2) + (j % 2)]
                ps = psS.tile([128, T], F32, tag="psS", name="psS")
                if g == 0 and j < 2:
                    # first-ever uses of the psS slots: make the dead band
                    # finite so the single exp below never sees raw psum
                    nc.vector.memset(ps[32:64, :], 0.0)
                for b in range(G):
                    bs = slice(49 * b, 49 * b + 49)
                    nc.tensor.matmul(ps[0:49, bs], k[j][0:64, bs],
                                     q[j][0:64, bs], start=True, stop=True)
                    nc.tensor.matmul(ps[64:113, bs], k[j][64:128, bs],
                                     q[j][64:128, bs], start=True, stop=True,
                                     tile_position=(64, 64))
                # ONE exp op: ACT cost scales with free size only, so a
                # single [0:113] pass costs half of two band passes. Rows
                # 49:64 hold stale-but-finite psum; their exps are killed
                # by onesbd zeros in the row-sum and never read via eTn.
                # (CoreSim reports NaN here — its fresh-tile memory has no
                # stale data — so this path is hardware-validated only.)
                nc.scalar.activation(eT[0:113, :], ps[0:113, :], EXP,
                                     scale=0.125)
                st["stash"][j] = [eT]

            def d_tail_a1(g, j):
                st = S[g]
                eT, = st["stash"][j]
                # per-pair base-0 psum tile: DVE lanes cannot shift
                # partitions, so the reciprocal must read rows 0:2
                pv_sum = psV.tile([2, T], F32, tag="psV", name="psV")
                nc.tensor.matmul(pv_sum[0:2, :], onesbd[:], eT[:],
                                 start=True, stop=True)
                st["stash"][j] = [eT, pv_sum]

            def d_tail_a2(g, j):
                st = S[g]
                eT, pv_sum = st["stash"][j]
                rr = p_rr.tile([2, T], F32, tag="rr", name="rr")
                nc.vector.reciprocal_approx_fast(rr[:], pv_sum[0:2, :])
                rrb = rrbs[j % 2]
                nc.vector.tensor_copy(rrb[0:2, :], rr[:])
                # partition_broadcast only reads partition 0, so move the
                # odd-head reciprocal (partition 1) to partition 0 of a
                # second tile via the DVE 32-lane shuffle
                rro = rros[j % 2]
                nc.vector.stream_shuffle(rro[:, :], rrb[:, :],
                                         mask=[1] + list(range(1, 32)))
                # per-token reciprocal broadcast across partitions on the
                # GPSIMD engine; output base partition must be 0 on HW, so
                # broadcast each head's reciprocal to all 128 partitions
                # and let the multiply read the matching half
                rbe = p_rb.tile([128, T], BF16, tag="rbE", name="rbE")
                nc.gpsimd.partition_broadcast(rbe[:, :], rrb[0:1, :],
                                              channels=128)
                rbo = p_rb.tile([128, T], BF16, tag="rbO", name="rbO")
                nc.gpsimd.partition_broadcast(rbo[:, :], rro[0:1, :],
                                              channels=128)
                # normalize the exp tile up front (a full double-step
                # before attn@V consumes it), so neither attn@V nor the
                # out-projection ever waits on the DVE/GPSIMD chain
                eTn = eTns[j % 2]
                nc.vector.tensor_mul(out=eTn[0:49, :], in0=eT[0:49, :],
                                     in1=rbe[0:49, :])
                nc.vector.tensor_mul(out=eTn[64:113, :], in0=eT[64:113, :],
                                     in1=rbo[64:113, :])
                st["stash"][j] = [eTn]

            def d_tail_b(g, j):
                st = S[g]
                eTn, = st["stash"][j]
                v4 = st["v4"]
                po = psO.tile([128, T], F32, tag="psO", name="psO")
                for b in range(G):
                    bs = slice(49 * b, 49 * b + 49)
                    nc.tensor.matmul(po[0:64, bs], v4[0:49, b, j, :],
                                     eTn[0:49, bs], start=True, stop=True)
                    nc.tensor.matmul(po[64:128, bs], v4[64:113, b, j, :],
                                     eTn[64:113, bs], start=True, stop=True,
                                     tile_position=(64, 64))
                st["stash"][j] = [po]

            def d_tail_c1(g, j):
                # ACT copy releases the po psum slot early (before E needs it)
                st = S[g]
                po, = st["stash"].pop(j)
                unT = st["unT"]
                nc.scalar.copy(unT[j][:], po[:])

            def emit_E(g, j2):
                st = S[g]
                unT = st["unT"]
                pp = psO.tile([128, T], F32, tag="psO", name="psO")
                for ci in range(6):
                    nc.tensor.matmul(
                        pp[:], wp6[:, ci, 128 * j2:128 * (j2 + 1)],
                        unT[ci][:], start=(ci == 0), stop=(ci == 5))
                osb = p_osb.tile([128, T], BF16, tag="osb", name="osb")
                # bias-add on ACT: it is the last reader of the pp psum
                # slot, and the ACT queue drains early each step, so psO
                # recycles in time for the next step's attn@V (a DVE
                # bias-add measured +53us: it gated psO from the DVE tail)
                nc.scalar.add(osb[:], pp[:], bias_sb[:, j2:j2 + 1])
                nc.sync.dma_start(out6[:, j2, g * T:(g + 1) * T],
                                  osb[:])

            step_idx = [0]

            # ---- bootstrap: group 0's B and C run standalone ----
            for jc in range(12):
                emit_B_chain(0, jc)
            load_xT(1)
            ensure_C(T)

            # ---- steady state: 3-stage pair pipeline. Step s emits, in
            # PE-queue order: attn@V of pair s-2 (consumes the normalized
            # exps produced a full step earlier, so it never waits on the
            # DVE/GPSIMD chain), the row-sums of pair s-1, the scores of
            # the current pair, then E units / next group's B and C. The
            # unT copies of pair s-2 go first on the ACT queue so the E
            # matmuls are never gated by them. The reciprocal + broadcast
            # + normalize chain of pair s-1 runs on DVE/GPSIMD with a
            # whole step of slack before attn@V reads its output. ----
            prev1 = None          # pair awaiting row-sum + normalize
            prev2 = None          # pair awaiting attn@V + unT copy
            e_queue = []          # (g, j2) E units awaiting emission

            def stage2(pg, p0, p1):
                d_tail_b(pg, p0)
                d_tail_b(pg, p1)
                d_tail_c1(pg, p0)
                d_tail_c1(pg, p1)
                if p1 == 5:
                    e_queue.extend((pg, j2) for j2 in range(6))

            def load_xT_part(g, jp):
                # spread the 2-group-ahead prefetch across the group's
                # three steps: issuing all 7 transfers at once bunches
                # ~5us of Sync-engine descriptor work right at the group
                # boundary, where the PE is most dependency-sensitive
                for ci in (2 * jp, 2 * jp + 1):
                    nc.sync.dma_start(
                        xv[:, ci, g * T:(g + 1) * T],
                        x6[:, ci, g * T:(g + 1) * T])
                if jp == 2:
                    nc.sync.dma_start(x8v[:, :, g * T:(g + 1) * T],
                                      x86[:, :, g * T:(g + 1) * T])

            for g in range(N_GROUPS):
                for jp in range(3):
                    if g + 2 < N_GROUPS:
                        load_xT_part(g + 2, jp)
                    if prev2:
                        stage2(*prev2)
                    if prev1:
                        d_tail_a1(prev1[0], prev1[1])
                        d_tail_a1(prev1[0], prev1[2])
                    d_head(g, 2 * jp)
                    d_head(g, 2 * jp + 1)
                    if prev1:
                        d_tail_a2(prev1[0], prev1[1])
                        d_tail_a2(prev1[0], prev1[2])
                    prev2, prev1 = prev1, (g, 2 * jp, 2 * jp + 1)
                    for _ in range(2):
                        if e_queue:
                            emit_E(*e_queue.pop(0))
                    if g + 1 < N_GROUPS:
                        for c4 in range(4):
                            emit_B_chain(g + 1, 4 * jp + c4)
                        ensure_C((g + 1) * T + (jp + 1) * T // 3)
                    step_idx[0] += 1

            # drain the two in-flight pairs, then the remaining E units
            for _ in range(2):
                if prev2:
                    stage2(*prev2)
                if prev1:
                    d_tail_a1(prev1[0], prev1[1])
                    d_tail_a1(prev1[0], prev1[2])
                    d_tail_a2(prev1[0], prev1[1])
                    d_tail_a2(prev1[0], prev1[2])
                prev2, prev1 = prev1, None
                for _ in range(2):
                    if e_queue:
                        emit_E(*e_queue.pop(0))
                step_idx[0] += 1
            for e in e_queue:
                emit_E(*e)

    nc.compile()
    return nc


def _prep_inputs(x, W_qkv, W_proj, b_proj):
    x = np.asarray(x, dtype=np.float32)
    B, N, Cc = x.shape
    consts = _consts()
    wq = np.asarray(W_qkv, dtype=np.float32)
    # permute W_v columns from (head, c) to (parity, pair, c) so the
    # C-stage GEMM emits even-head features in cols 0:384 and odd-head
    # in 384:768 (contiguous copy + contiguous scatter on device)
    wv = wq[:, 2 * Cc:3 * Cc].reshape(Cc, 6, 2, 64)
    wv = np.ascontiguousarray(wv.transpose(0, 2, 1, 3)).reshape(Cc, Cc)
    wqv = np.ascontiguousarray(
        np.concatenate([wq[:, 0:Cc], wv], axis=1)).astype(BF)
    wk8 = np.ascontiguousarray(wq[:, Cc:2 * Cc]).astype(F8)
    wproj = np.ascontiguousarray(np.asarray(W_proj, dtype=np.float32)).astype(BF)
    bias = np.ascontiguousarray(
        np.asarray(b_proj, dtype=np.float32).reshape(6, 128).T)
    x_bf = x.astype(BF)
    x_f8 = x.astype(F8)
    in_maps = []
    def tile_pm(a):
        # [C, TOK] feature-major -> partition-major [128, 6*TOK] matching
        # the SBUF layout run-for-run (c outer, p inner, as in "(c p) t")
        return np.ascontiguousarray(
            a.reshape(6, 128, TOK).transpose(1, 0, 2).reshape(128, 6 * TOK))

    for i in range(NUM_CORES):
        xt = tile_pm(x_bf[i * B_CORE:(i + 1) * B_CORE].reshape(TOK, Cc).T)
        xt8 = tile_pm(x_f8[i * B_CORE:(i + 1) * B_CORE].reshape(TOK, Cc).T)
        m = {"x": xt, "x8": xt8, "wqv": wqv, "wk8": wk8, "wproj": wproj,
             "bias": bias}
        m.update(consts)
        in_maps.append(m)
    return in_maps


def _unshard(results):
    out = np.empty((NUM_CORES * B_CORE, SEQ, C), dtype=np.float32)
    for i in range(NUM_CORES):
        o = np.asarray(results[i]["out"]).astype(np.float32)  # [128, 6*TOK]
        o = o.reshape(128, 6, TOK).transpose(1, 0, 2).reshape(C, TOK)
        out[i * B_CORE:(i + 1) * B_CORE] = o.T.reshape(B_CORE, SEQ, C)
    return out


def kernel(x, W_qkv, W_proj, b_proj):
    from concourse.bass_utils import run_bass_kernel_spmd

    if "nc" not in _CACHE:
        _CACHE["nc"] = _build()
    nc = _CACHE["nc"]

    in_maps = _prep_inputs(x, W_qkv, W_proj, b_proj)
    res = run_bass_kernel_spmd(nc, in_maps, list(range(NUM_CORES)))
    return _unshard(res.results)


# revision 56
# speedup vs baseline: 1.0937x; 1.0937x over previous
"""Trainium2 Bass kernel: fused attention block (QKV proj -> MHA -> out proj).

Reference (per batch item b, NUM_HEADS=12, Dh=64):
    qkv = x @ W_qkv; q,k,v per head
    attn = softmax(q @ k^T / 8) @ v
    out  = concat_heads(attn) @ W_proj + b_proj

Sharding: data-parallel over batch across 8 NeuronCores (128 batch items
per core), weights replicated. One SPMD Bass program, per-core inputs.

Design (bf16 + fp8 k-chains, feature-major I/O, half-array head pairing,
fully software-pipelined):
  - Host pre-transposes x to feature-major [C, TOK] bf16 (and an fp8
    copy for the k-projection) and transposes the feature-major bf16
    output back; device never transposes.
  - Groups of G=8 batches (T=392 tokens). Per group:
    B: q co-tiles [128, T] = Wq_slice.T @ xT in bf16; k co-tiles via
       fp8 DoubleRow matmuls (2 c-chunks contracted per instruction at
       2x rate) from the fp8 copies of x and W_k. Scores tolerate the
       one-sided fp8 noise (k fp8, q bf16: rel err 1.4e-2 < 2e-2).
    C: v token-major in 4 tiles of 98 tokens, scattered by SBUF->SBUF
       DMA into per-(head-pair j, batch b) blocks vbd2 [128, 64]:
       rows 0:49 = even-head key positions, 64:113 = odd-head.
    D: per j: 8 batches x 2 half-array matmuls (even head in array
       quadrants (0:64)x(0:64), odd head at tile_position=(64,64))
       for scores and attn@V; single-op exp on ACT; row sums via ones
       matmul; reciprocal on DVE; the per-token reciprocal is
       broadcast across partitions on the GPSIMD engine
       (partition_broadcast) instead of a tensor-engine broadcast
       matmul, keeping the PE free for real GEMM work.
    E: out co-tiles [128, T] = Wproj_slice.T @ unT + per-partition bias.
  - Steady state runs D in head-pair double-steps and interleaves E of
    the previous group plus B and C of the next group into them, so the
    tensor engine sees one dense instruction stream and the HAM clock
    gate stays at full rate.
"""
import sys

sys.path.insert(0, "/opt/trn_rl_repo")

import numpy as np
import ml_dtypes

NUM_CORES = 8
B_CORE = 128          # batch items per core
SEQ = 49              # tokens per batch item
C = 768               # channels
H = 12                # heads
G = 8                 # batch items per group
T = SEQ * G           # 392 tokens per group
TOK = B_CORE * SEQ    # 6272 tokens per core
N_GROUPS = B_CORE // G

BF = ml_dtypes.bfloat16
F8 = ml_dtypes.float8_e4m3fn

_CACHE = {}


def _consts():
    onesbd = np.zeros((128, 2), dtype=BF)
    onesbd[0:49, 0] = 1.0
    onesbd[64:113, 1] = 1.0
    return {"onesbd": onesbd}


def _build():
    import concourse.bacc as bacc
    import concourse.mybir as mybir
    import concourse.tile as tile

    F32 = mybir.dt.float32
    BF16 = mybir.dt.bfloat16
    FP8 = mybir.dt.float8e4
    EXP = mybir.ActivationFunctionType.Exp
    DR = mybir.MatmulPerfMode.DoubleRow

    nc = bacc.Bacc("TRN2", target_bir_lowering=False)

    # x/x8/out are host-tiled to partition-major [128, 6*TOK] so every
    # DMA's source run structure matches the SBUF destination exactly
    # (collapses Sync-engine descriptor generation)
    d_x = nc.declare_dram_parameter("x", [128, 6 * TOK], BF16,
                                    isOutput=False)
    d_x8 = nc.declare_dram_parameter("x8", [128, 6 * TOK], FP8,
                                     isOutput=False)
    # bf16 weights: q columns (0:C) and v columns (C:2C of this tensor)
    d_wqv = nc.declare_dram_parameter("wqv", [C, 2 * C], BF16, isOutput=False)
    d_wk8 = nc.declare_dram_parameter("wk8", [C, C], FP8, isOutput=False)
    d_wproj = nc.declare_dram_parameter("wproj", [C, C], BF16, isOutput=False)
    d_bias = nc.declare_dram_parameter("bias", [128, 6], F32, isOutput=False)
    d_onesbd = nc.declare_dram_parameter("onesbd", [128, 2], BF16,
                                         isOutput=False)
    d_out = nc.declare_dram_parameter("out", [128, 6 * TOK], BF16,
                                      isOutput=True)

    # DRAM views with the 6x128 channel-tile structure exposed, so one DMA
    # moves all 6 channel tiles of a slice
    x6 = d_x.rearrange("p (c t) -> p c t", t=TOK)
    x86 = d_x8.rearrange("p (c t) -> p c t", t=TOK)
    out6 = d_out.rearrange("p (c t) -> p c t", t=TOK)
    wqv6 = d_wqv.rearrange("(c p) n -> p c n", p=128)
    wk86 = d_wk8.rearrange("(c p) n -> p c n", p=128)
    wproj6 = d_wproj.rearrange("(c p) n -> p c n", p=128)

    with tile.TileContext(nc) as tc, \
         nc.allow_low_precision(reason="bf16/fp8 matmuls within 2e-2 tol"):
        with tc.tile_pool(name="wres", bufs=1) as wres, \
             tc.tile_pool(name="qk", bufs=2) as p_qk, \
             tc.tile_pool(name="scr", bufs=2) as p_scr, \
             tc.tile_pool(name="vbd2", bufs=3) as p_vbd2, \
             tc.tile_pool(name="rr", bufs=2) as p_rr, \
             tc.tile_pool(name="rb", bufs=2) as p_rb, \
             tc.tile_pool(name="unT", bufs=2) as p_unT, \
             tc.tile_pool(name="osb", bufs=2) as p_osb, \
             tc.tile_pool(name="psA", bufs=2, space="PSUM") as psA, \
             tc.tile_pool(name="psS", bufs=2, space="PSUM") as psS, \
             tc.tile_pool(name="psO", bufs=2, space="PSUM") as psO, \
             tc.tile_pool(name="psV", bufs=2, space="PSUM") as psV:

            # ---- resident weights / constants ----
            # q weight columns first: the first B-stage matmuls need
            # only these plus the group-0 x tiles
            w_qv = wres.tile([128, 6 * 2 * C], BF16, tag="wqv", name="wqv")
            wq6 = w_qv.rearrange("p (c n) -> p c n", n=2 * C)
            # critical-path-first startup: the first B chain needs only
            # wq cols 0:128 plus x chunk ci=0, so issue those ahead of
            # the remaining ~2MB of weights
            nc.sync.dma_start(wq6[:, :, 0:128], wqv6[:, :, 0:128])

            S = [dict() for _ in range(N_GROUPS)]

            # x resident feature-major for the whole core: B slices it per
            # group, C slices it in clean 128-token tiles across group
            # boundaries (no runt tiles, full partition utilization).
            # x8 is the fp8 copy feeding the DoubleRow k-chains.
            xTall = wres.tile([128, 6 * TOK], BF16, tag="xTall", name="xTall")
            xv = xTall.rearrange("p (c t) -> p c t", t=TOK)
            x8all = wres.tile([128, 6 * TOK], FP8, tag="x8all", name="x8all")
            x8v = x8all.rearrange("p (c t) -> p c t", t=TOK)

            def load_xT(g, by_ci=False, x8=True):
                if by_ci:
                    # per-channel-chunk loads: the first B chain's ci-th
                    # matmul can start as soon as chunk ci has landed
                    for ci in range(6):
                        nc.sync.dma_start(
                            xv[:, ci, g * T:(g + 1) * T],
                            x6[:, ci, g * T:(g + 1) * T])
                else:
                    nc.sync.dma_start(
                        xv[:, :, g * T:(g + 1) * T],
                        x6[:, :, g * T:(g + 1) * T])
                if x8:
                    nc.sync.dma_start(x8v[:, :, g * T:(g + 1) * T],
                                      x86[:, :, g * T:(g + 1) * T])

            load_xT(0, by_ci=True, x8=False)
            # rest of the q weights, then the fp8 copies (k chains run
            # after all six q chains in the bootstrap)
            for o in range(128, C, 128):
                nc.sync.dma_start(wq6[:, :, o:o + 128],
                                  wqv6[:, :, o:o + 128])
            nc.sync.dma_start(x8v[:, :, 0:T], x86[:, :, 0:T])
            # fp8 k weights (needed by the k chains of group 0)
            wk8 = wres.tile([128, 6 * C], FP8, tag="wk8", name="wk8")
            wk6 = wk8.rearrange("p (c n) -> p c n", n=C)
            for o in range(0, C, 384):
                nc.sync.dma_start(wk6[:, :, o:o + 384],
                                  wk86[:, :, o:o + 384])
            # v weight columns (C stage)
            for o in range(C, 2 * C, 384):
                nc.sync.dma_start(wq6[:, :, o:o + 384],
                                  wqv6[:, :, o:o + 384])
            w_proj = wres.tile([128, 6 * C], BF16, tag="wproj", name="wproj")
            wp6 = w_proj.rearrange("p (c n) -> p c n", n=C)
            for o in range(0, C, 384):
                nc.sync.dma_start(wp6[:, :, o:o + 384],
                                  wproj6[:, :, o:o + 384])
            onesbd = wres.tile([128, 2], BF16, tag="onesbd")
            nc.sync.dma_start(onesbd[:], d_onesbd[:])
            bias_sb = wres.tile([128, 6], F32, tag="bias_sb")
            nc.sync.dma_start(bias_sb[:], d_bias[:])
            # exp tiles: dead bands (rows 49:64, 113:128) must stay zero.
            # 4 buffers: the tail pipeline is 2 steps deep, and each step
            # holds a pair (even j, odd j), so a tile written at step s is
            # still read at s+1 and can only be reused at s+2.
            eTs = []
            for nm in ("eTa0", "eTb0", "eTa1", "eTb1"):
                t = wres.tile([128, T], BF16, tag=nm, name=nm)
                nc.vector.memset(t[:], 0.0)
                eTs.append(t)
            # normalized exp tiles (only rows 0:49 / 64:113 ever read)
            eTns = []
            for nm in ("eTnA", "eTnB"):
                t = wres.tile([128, T], BF16, tag=nm, name=nm)
                eTns.append(t)
            # persistent double-buffered reciprocal staging tiles: the
            # 32-lane shuffle reads all 32 partitions, so they must be
            # fully initialized once (rows 2:32 stay at the memset value)
            rrbs, rros = [], []
            for nm in ("rrbA", "rrbB"):
                t = wres.tile([32, T], BF16, tag=nm, name=nm)
                nc.vector.memset(t[:], 0.0)
                rrbs.append(t)
            for nm in ("rroA", "rroB"):
                t = wres.tile([32, T], BF16, tag=nm, name=nm)
                nc.vector.memset(t[:], 0.0)
                rros.append(t)

            # ---- emission helpers ----
            def emit_B_chain(g, jc):
                st = S[g]
                if jc == 0:
                    st["q"], st["k"] = [None] * 6, [None] * 6
                pq = psA.tile([128, T], F32, tag="psA", name="psA")
                if jc < 6:
                    for ci in range(6):
                        nc.tensor.matmul(
                            pq[:], wq6[:, ci, 128 * jc:128 * (jc + 1)],
                            xv[:, ci, g * T:(g + 1) * T],
                            start=(ci == 0), stop=(ci == 5))
                else:
                    # fp8 DoubleRow: contract two c-chunks per matmul at
                    # 2x rate. 196-col halves keep the moving AP free dim
                    # at 392 <= 512 (the fast path). Both halves form ONE
                    # accumulation group: only the very first matmul sets
                    # start, so the bank's pending-zero covers the second
                    # half's columns instead of being re-marked (which
                    # would wipe the first half's results).
                    jk = jc - 6
                    for hh in range(2):
                        ts0 = g * T + 196 * hh
                        for c3 in range(3):
                            nc.tensor.matmul(
                                pq[:, 196 * hh:196 * (hh + 1)],
                                wk6[:, 2 * c3:2 * c3 + 2,
                                    128 * jk:128 * (jk + 1)],
                                x8v[:, 2 * c3:2 * c3 + 2, ts0:ts0 + 196],
                                start=(hh == 0 and c3 == 0),
                                stop=(hh == 1 and c3 == 2),
                                perf_mode=DR)
                nm = f"q{jc}" if jc < 6 else f"k{jc - 6}"
                t = p_qk.tile([128, T], BF16, tag=nm, name=nm)
                # all chain copies on ACT: GPSIMD cannot access PSUM
                # (BIR rule), and a DVE copy queues behind the deep
                # recip/shuffle/normalize chain, gating psA rotation
                nc.scalar.copy(t[:], pq[:])
                if jc < 6:
                    st["q"][jc] = t
                else:
                    st["k"][jc - 6] = t

            c_next = [0]

            def emit_C_unit():
                t4 = c_next[0]
                c_next[0] += 1
                tok0 = 128 * t4
                # scr layout [p, (parity, j, c)]: even-head features in
                # cols 0:384, odd in 384:768, so each scatter side is one
                # contiguous 384-element run per partition (cheap DMA
                # descriptor generation on the Sync engine)
                # host permuted the W_v columns to (parity, pair, c) order,
                # so half 0 of the GEMM output is all even-head features
                # and half 1 all odd-head: the copy is contiguous and each
                # scatter side is one 384-element run per partition
                scr = p_scr.tile([128, C], BF16, tag="scr", name="scr")
                for half in range(2):
                    pv = psS.tile([128, 384], F32, tag="psS", name="psS")
                    for ci in range(6):
                        nc.tensor.matmul(
                            pv[:], xv[:, ci, tok0:tok0 + 128],
                            wq6[:, ci, 768 + 384 * half:
                                768 + 384 * (half + 1)],
                            start=(ci == 0), stop=(ci == 5))
                    nc.vector.tensor_copy(
                        scr[:, 384 * half:384 * (half + 1)], pv[:])
                for b in range(tok0 // SEQ, min(B_CORE, (tok0 + 127) // SEQ + 1)):
                    lo = max(SEQ * b, tok0)
                    hi = min(SEQ * (b + 1), tok0 + 128)
                    if lo >= hi:
                        continue
                    gb, bb = b // G, b % G
                    if "v4" not in S[gb]:
                        vbd2 = p_vbd2.tile([128, G * 6 * 64], BF16,
                                           tag="vbd2", name="vbd2")
                        # [p, b, j, c]: per-(batch) 384-element contiguous
                        S[gb]["v4"] = vbd2.rearrange(
                            "p (b j c) -> p b j c", b=G, c=64)
                        S[gb]["v4f"] = vbd2.rearrange(
                            "p (b n) -> p b n", b=G)
                    v4f = S[gb]["v4f"]
                    sl, sh = lo - SEQ * b, hi - SEQ * b
                    nc.sync.dma_start(v4f[sl:sh, bb, :],
                                      scr[lo - tok0:hi - tok0, 0:384])
                    nc.sync.dma_start(v4f[64 + sl:64 + sh, bb, :],
                                      scr[lo - tok0:hi - tok0, 384:768])

            def ensure_C(tok_thresh):
                while c_next[0] < TOK // 128 and 128 * c_next[0] < tok_thresh:
                    emit_C_unit()

            def d_head(g, j):
                st = S[g]
                if j == 0:
                    st["unT"] = [p_unT.tile([128, T], BF16, tag=f"unT{ci}",
                                            name=f"unT{ci}")
                                 for ci in range(6)]
                    st["stash"] = {}
                q, k = st["q"], st["k"]
                eT = eTs[2 * (step_idx[0] % 2) + (j % 2)]
                ps = psS.tile([128, T], F32, tag="psS", name="psS")
                if g == 0 and j < 2:
                    # first-ever uses of the psS slots: make the dead band
                    # finite so the single exp below never sees raw psum
                    nc.vector.memset(ps[32:64, :], 0.0)
                for b in range(G):
                    bs = slice(49 * b, 49 * b + 49)
                    nc.tensor.matmul(ps[0:49, bs], k[j][0:64, bs],
                                     q[j][0:64, bs], start=True, stop=True)
                    nc.tensor.matmul(ps[64:113, bs], k[j][64:128, bs],
                                     q[j][64:128, bs], start=True, stop=True,
                                     tile_position=(64, 64))
                # ONE exp op: ACT cost scales with free size only, so a
                # single [0:113] pass costs half of two band passes. Rows
                # 49:64 hold stale-but-finite psum; their exps are killed
                # by onesbd zeros in the row-sum and never read via eTn.
                # (CoreSim reports NaN here — its fresh-tile memory has no
                # stale data — so this path is hardware-validated only.)
                nc.scalar.activation(eT[0:113, :], ps[0:113, :], EXP,
                                     scale=0.125)
                st["stash"][j] = [eT]

            def d_tail_a1(g, j):
                st = S[g]
                eT, = st["stash"][j]
                # per-pair base-0 psum tile: DVE lanes cannot shift
                # partitions, so the reciprocal must read rows 0:2
                pv_sum = psV.tile([2, T], F32, tag="psV", name="psV")
                nc.tensor.matmul(pv_sum[0:2, :], onesbd[:], eT[:],
                                 start=True, stop=True)
                st["stash"][j] = [eT, pv_sum]

            def d_tail_a2(g, j):
                st = S[g]
                eT, pv_sum = st["stash"][j]
                rr = p_rr.tile([2, T], F32, tag="rr", name="rr")
                nc.vector.reciprocal_approx_fast(rr[:], pv_sum[0:2, :])
                rrb = rrbs[j % 2]
                nc.vector.tensor_copy(rrb[0:2, :], rr[:])
                # partition_broadcast only reads partition 0, so move the
                # odd-head reciprocal (partition 1) to partition 0 of a
                # second tile via the DVE 32-lane shuffle
                rro = rros[j % 2]
                nc.vector.stream_shuffle(rro[:, :], rrb[:, :],
                                         mask=[1] + list(range(1, 32)))
                # per-token reciprocal broadcast across partitions on the
                # GPSIMD engine; output base partition must be 0 on HW, so
                # broadcast each head's reciprocal to all 128 partitions
                # and let the multiply read the matching half
                rbe = p_rb.tile([128, T], BF16, tag="rbE", name="rbE")
                nc.gpsimd.partition_broadcast(rbe[:, :], rrb[0:1, :],
                                              channels=128)
                rbo = p_rb.tile([128, T], BF16, tag="rbO", name="rbO")
                nc.gpsimd.partition_broadcast(rbo[:, :], rro[0:1, :],
                                              channels=128)
                # normalize the exp tile up front (a full double-step
                # before attn@V consumes it), so neither attn@V nor the
                # out-projection ever waits on the DVE/GPSIMD chain
                eTn = eTns[j % 2]
                nc.vector.tensor_mul(out=eTn[0:49, :], in0=eT[0:49, :],
                                     in1=rbe[0:49, :])
                nc.vector.tensor_mul(out=eTn[64:113, :], in0=eT[64:113, :],
                                     in1=rbo[64:113, :])
                st["stash"][j] = [eTn]

            def d_tail_b(g, j):
                st = S[g]
                eTn, = st["stash"][j]
                v4 = st["v4"]
                po = psO.tile([128, T], F32, tag="psO", name="psO")
                for b in range(G):
                    bs = slice(49 * b, 49 * b + 49)
                    nc.tensor.matmul(po[0:64, bs], v4[0:49, b, j, :],
                                     eTn[0:49, bs], start=True, stop=True)
                    nc.tensor.matmul(po[64:128, bs], v4[64:113, b, j, :],
                                     eTn[64:113, bs], start=True, stop=True,
                                     tile_position=(64, 64))
                st["stash"][j] = [po]

            def d_tail_c1(g, j):
                # ACT copy releases the po psum slot early (before E needs it)
                st = S[g]
                po, = st["stash"].pop(j)
                unT = st["unT"]
                nc.scalar.copy(unT[j][:], po[:])

            def emit_E(g, j2):
                st = S[g]
                unT = st["unT"]
                pp = psO.tile([128, T], F32, tag="psO", name="psO")
                for ci in range(6):
                    nc.tensor.matmul(
                        pp[:], wp6[:, ci, 128 * j2:128 * (j2 + 1)],
                        unT[ci][:], start=(ci == 0), stop=(ci == 5))
                osb = p_osb.tile([128, T], BF16, tag="osb", name="osb")
                # bias-add on ACT: it is the last reader of the pp psum
                # slot, and the ACT queue drains early each step, so psO
                # recycles in time for the next step's attn@V (a DVE
                # bias-add measured +53us: it gated psO from the DVE tail)
                nc.scalar.add(osb[:], pp[:], bias_sb[:, j2:j2 + 1])
                nc.sync.dma_start(out6[:, j2, g * T:(g + 1) * T],
                                  osb[:])

            step_idx = [0]

            # ---- bootstrap: group 0's B and C run standalone ----
            for jc in range(12):
                emit_B_chain(0, jc)
            load_xT(1)
            ensure_C(T)

            # ---- steady state: 3-stage pair pipeline. Step s emits, in
            # PE-queue order: attn@V of pair s-2 (consumes the normalized
            # exps produced a full step earlier, so it never waits on the
            # DVE/GPSIMD chain), the row-sums of pair s-1, the scores of
            # the current pair, then E units / next group's B and C. The
            # unT copies of pair s-2 go first on the ACT queue so the E
            # matmuls are never gated by them. The reciprocal + broadcast
            # + normalize chain of pair s-1 runs on DVE/GPSIMD with a
            # whole step of slack before attn@V reads its output. ----
            prev1 = None          # pair awaiting row-sum + normalize
            prev2 = None          # pair awaiting attn@V + unT copy
            e_queue = []          # (g, j2) E units awaiting emission

            def stage2(pg, p0, p1):
                d_tail_b(pg, p0)
                d_tail_b(pg, p1)
                d_tail_c1(pg, p0)
                d_tail_c1(pg, p1)
                if p1 == 5:
                    e_queue.extend((pg, j2) for j2 in range(6))

            for g in range(N_GROUPS):
                if g + 2 < N_GROUPS:
                    load_xT(g + 2)
                for jp in range(3):
                    if prev2:
                        stage2(*prev2)
                    if prev1:
                        d_tail_a1(prev1[0], prev1[1])
                        d_tail_a1(prev1[0], prev1[2])
                    d_head(g, 2 * jp)
                    d_head(g, 2 * jp + 1)
                    if prev1:
                        d_tail_a2(prev1[0], prev1[1])
                        d_tail_a2(prev1[0], prev1[2])
                    prev2, prev1 = prev1, (g, 2 * jp, 2 * jp + 1)
                    for _ in range(2):
                        if e_queue:
                            emit_E(*e_queue.pop(0))
                    if g + 1 < N_GROUPS:
                        for c4 in range(4):
                            emit_B_chain(g + 1, 4 * jp + c4)
                        ensure_C((g + 1) * T + (jp + 1) * T // 3)
                    step_idx[0] += 1

            # drain the two in-flight pairs, then the remaining E units
            for _ in range(2):
                if prev2:
                    stage2(*prev2)
                if prev1:
                    d_tail_a1(prev1[0], prev1[1])
                    d_tail_a1(prev1[0], prev1[2])
                    d_tail_a2(prev1[0], prev1[1])
                    d_tail_a2(prev1[0], prev1[2])
                prev2, prev1 = prev1, None
                for _ in range(2):
                    if e_queue:
                        emit_E(*e_queue.pop(0))
                step_idx[0] += 1
            for e in e_queue:
                emit_E(*e)

    nc.compile()
    return nc


def _prep_inputs(x, W_qkv, W_proj, b_proj):
    x = np.asarray(x, dtype=np.float32)
    B, N, Cc = x.shape
    consts = _consts()
    wq = np.asarray(W_qkv, dtype=np.float32)
    # permute W_v columns from (head, c) to (parity, pair, c) so the
    # C-stage GEMM emits even-head features in cols 0:384 and odd-head
    # in 384:768 (contiguous copy + contiguous scatter on device)
    wv = wq[:, 2 * Cc:3 * Cc].reshape(Cc, 6, 2, 64)
    wv = np.ascontiguousarray(wv.transpose(0, 2, 1, 3)).reshape(Cc, Cc)
    wqv = np.ascontiguousarray(
        np.concatenate([wq[:, 0:Cc], wv], axis=1)).astype(BF)
    wk8 = np.ascontiguousarray(wq[:, Cc:2 * Cc]).astype(F8)
    wproj = np.ascontiguousarray(np.asarray(W_proj, dtype=np.float32)).astype(BF)
    bias = np.ascontiguousarray(
        np.asarray(b_proj, dtype=np.float32).reshape(6, 128).T)
    x_bf = x.astype(BF)
    x_f8 = x.astype(F8)
    in_maps = []
    def tile_pm(a):
        # [C, TOK] feature-major -> partition-major [128, 6*TOK] matching
        # the SBUF layout run-for-run (c outer, p inner, as in "(c p) t")
        return np.ascontiguousarray(
            a.reshape(6, 128, TOK).transpose(1, 0, 2).reshape(128, 6 * TOK))

    for i in range(NUM_CORES):
        xt = tile_pm(x_bf[i * B_CORE:(i + 1) * B_CORE].reshape(TOK, Cc).T)
        xt8 = tile_pm(x_f8[i * B_CORE:(i + 1) * B_CORE].reshape(TOK, Cc).T)
        m = {"x": xt, "x8": xt8, "wqv": wqv, "wk8": wk8, "wproj": wproj,
             "bias": bias}
        m.update(consts)
        in_maps.append(m)
    return in_maps


def _unshard(results):
    out = np.empty((NUM_CORES * B_CORE, SEQ, C), dtype=np.float32)
    for i in range(NUM_CORES):
        o = np.asarray(results[i]["out"]).astype(np.float32)  # [128, 6*TOK]
        o = o.reshape(128, 6, TOK).transpose(1, 0, 2).reshape(C, TOK)
        out[i * B_CORE:(i + 1) * B_CORE] = o.T.reshape(B_CORE, SEQ, C)
    return out


def kernel(x, W_qkv, W_proj, b_proj):
    from concourse.bass_utils import run_bass_kernel_spmd

    if "nc" not in _CACHE:
        _CACHE["nc"] = _build()
    nc = _CACHE["nc"]

    in_maps = _prep_inputs(x, W_qkv, W_proj, b_proj)
    res = run_bass_kernel_spmd(nc, in_maps, list(range(NUM_CORES)))
    return _unshard(res.results)


# revision 59
# speedup vs baseline: 1.1040x; 1.0094x over previous
"""Trainium2 Bass kernel: fused attention block (QKV proj -> MHA -> out proj).

Reference (per batch item b, NUM_HEADS=12, Dh=64):
    qkv = x @ W_qkv; q,k,v per head
    attn = softmax(q @ k^T / 8) @ v
    out  = concat_heads(attn) @ W_proj + b_proj

Sharding: data-parallel over batch across 8 NeuronCores (128 batch items
per core), weights replicated. One SPMD Bass program, per-core inputs.

Design (bf16 + fp8 k-chains, feature-major I/O, half-array head pairing,
fully software-pipelined):
  - Host pre-transposes x to feature-major [C, TOK] bf16 (and an fp8
    copy for the k-projection) and transposes the feature-major bf16
    output back; device never transposes.
  - Groups of G=8 batches (T=392 tokens). Per group:
    B: q co-tiles [128, T] = Wq_slice.T @ xT in bf16; k co-tiles via
       fp8 DoubleRow matmuls (2 c-chunks contracted per instruction at
       2x rate) from the fp8 copies of x and W_k. Scores tolerate the
       one-sided fp8 noise (k fp8, q bf16: rel err 1.4e-2 < 2e-2).
    C: v token-major in 4 tiles of 98 tokens, scattered by SBUF->SBUF
       DMA into per-(head-pair j, batch b) blocks vbd2 [128, 64]:
       rows 0:49 = even-head key positions, 64:113 = odd-head.
    D: per j: 8 batches x 2 half-array matmuls (even head in array
       quadrants (0:64)x(0:64), odd head at tile_position=(64,64))
       for scores and attn@V; single-op exp on ACT; row sums via ones
       matmul; reciprocal on DVE; the per-token reciprocal is
       broadcast across partitions on the GPSIMD engine
       (partition_broadcast) instead of a tensor-engine broadcast
       matmul, keeping the PE free for real GEMM work.
    E: out co-tiles [128, T] = Wproj_slice.T @ unT + per-partition bias.
  - Steady state runs D in head-pair double-steps and interleaves E of
    the previous group plus B and C of the next group into them, so the
    tensor engine sees one dense instruction stream and the HAM clock
    gate stays at full rate.
"""
import sys

sys.path.insert(0, "/opt/trn_rl_repo")

import numpy as np
import ml_dtypes

NUM_CORES = 8
B_CORE = 128          # batch items per core
SEQ = 49              # tokens per batch item
C = 768               # channels
H = 12                # heads
G = 8                 # batch items per group
T = SEQ * G           # 392 tokens per group
TOK = B_CORE * SEQ    # 6272 tokens per core
N_GROUPS = B_CORE // G

BF = ml_dtypes.bfloat16
F8 = ml_dtypes.float8_e4m3fn

_CACHE = {}


def _consts():
    onesbd = np.zeros((128, 2), dtype=BF)
    onesbd[0:49, 0] = 1.0
    onesbd[64:113, 1] = 1.0
    return {"onesbd": onesbd}


def _build():
    import concourse.bacc as bacc
    import concourse.mybir as mybir
    import concourse.tile as tile

    F32 = mybir.dt.float32
    BF16 = mybir.dt.bfloat16
    FP8 = mybir.dt.float8e4
    EXP = mybir.ActivationFunctionType.Exp
    DR = mybir.MatmulPerfMode.DoubleRow

    nc = bacc.Bacc("TRN2", target_bir_lowering=False)

    # x/x8/out are host-tiled to partition-major [128, 6*TOK] so every
    # DMA's source run structure matches the SBUF destination exactly
    # (collapses Sync-engine descriptor generation)
    d_x = nc.declare_dram_parameter("x", [128, 6 * TOK], BF16,
                                    isOutput=False)
    d_x8 = nc.declare_dram_parameter("x8", [128, 6 * TOK], FP8,
                                     isOutput=False)
    # bf16 weights: q columns (0:C) and v columns (C:2C of this tensor);
    # all weights host-tiled to partition-major [128, 6*N] like x, so
    # their startup DMAs are run-contiguous on both sides
    d_wqv = nc.declare_dram_parameter("wqv", [128, 6 * 2 * C], BF16,
                                      isOutput=False)
    d_wk8 = nc.declare_dram_parameter("wk8", [128, 6 * C], FP8,
                                      isOutput=False)
    d_wproj = nc.declare_dram_parameter("wproj", [128, 6 * C], BF16,
                                        isOutput=False)
    d_bias = nc.declare_dram_parameter("bias", [128, 6], F32, isOutput=False)
    d_onesbd = nc.declare_dram_parameter("onesbd", [128, 2], BF16,
                                         isOutput=False)
    d_out = nc.declare_dram_parameter("out", [128, 6 * TOK], BF16,
                                      isOutput=True)

    # DRAM views with the 6x128 channel-tile structure exposed, so one DMA
    # moves all 6 channel tiles of a slice
    x6 = d_x.rearrange("p (c t) -> p c t", t=TOK)
    x86 = d_x8.rearrange("p (c t) -> p c t", t=TOK)
    out6 = d_out.rearrange("p (c t) -> p c t", t=TOK)
    wqv6 = d_wqv.rearrange("p (c n) -> p c n", n=2 * C)
    wk86 = d_wk8.rearrange("p (c n) -> p c n", n=C)
    wproj6 = d_wproj.rearrange("p (c n) -> p c n", n=C)

    with tile.TileContext(nc) as tc, \
         nc.allow_low_precision(reason="bf16/fp8 matmuls within 2e-2 tol"):
        with tc.tile_pool(name="wres", bufs=1) as wres, \
             tc.tile_pool(name="qk", bufs=2) as p_qk, \
             tc.tile_pool(name="scr", bufs=2) as p_scr, \
             tc.tile_pool(name="vbd2", bufs=3) as p_vbd2, \
             tc.tile_pool(name="rr", bufs=2) as p_rr, \
             tc.tile_pool(name="rb", bufs=2) as p_rb, \
             tc.tile_pool(name="unT", bufs=2) as p_unT, \
             tc.tile_pool(name="osb", bufs=2) as p_osb, \
             tc.tile_pool(name="psA", bufs=2, space="PSUM") as psA, \
             tc.tile_pool(name="psS", bufs=2, space="PSUM") as psS, \
             tc.tile_pool(name="psO", bufs=2, space="PSUM") as psO, \
             tc.tile_pool(name="psV", bufs=2, space="PSUM") as psV:

            # ---- resident weights / constants ----
            # q weight columns first: the first B-stage matmuls need
            # only these plus the group-0 x tiles
            w_qv = wres.tile([128, 6 * 2 * C], BF16, tag="wqv", name="wqv")
            wq6 = w_qv.rearrange("p (c n) -> p c n", n=2 * C)
            # critical-path-first startup: the first B chain needs only
            # wq cols 0:128 plus x chunk ci=0, so issue those ahead of
            # the remaining ~2MB of weights
            nc.sync.dma_start(wq6[:, :, 0:128], wqv6[:, :, 0:128])

            S = [dict() for _ in range(N_GROUPS)]

            # x resident feature-major for the whole core: B slices it per
            # group, C slices it in clean 128-token tiles across group
            # boundaries (no runt tiles, full partition utilization).
            # x8 is the fp8 copy feeding the DoubleRow k-chains.
            xTall = wres.tile([128, 6 * TOK], BF16, tag="xTall", name="xTall")
            xv = xTall.rearrange("p (c t) -> p c t", t=TOK)
            x8all = wres.tile([128, 6 * TOK], FP8, tag="x8all", name="x8all")
            x8v = x8all.rearrange("p (c t) -> p c t", t=TOK)

            def load_xT(g, by_ci=False, x8=True):
                if by_ci:
                    # per-channel-chunk loads: the first B chain's ci-th
                    # matmul can start as soon as chunk ci has landed
                    for ci in range(6):
                        nc.sync.dma_start(
                            xv[:, ci, g * T:(g + 1) * T],
                            x6[:, ci, g * T:(g + 1) * T])
                else:
                    nc.sync.dma_start(
                        xv[:, :, g * T:(g + 1) * T],
                        x6[:, :, g * T:(g + 1) * T])
                if x8:
                    nc.sync.dma_start(x8v[:, :, g * T:(g + 1) * T],
                                      x86[:, :, g * T:(g + 1) * T])

            load_xT(0, by_ci=True, x8=False)
            # rest of the q weights, then the fp8 copies (k chains run
            # after all six q chains in the bootstrap)
            for o in range(128, C, 128):
                nc.sync.dma_start(wq6[:, :, o:o + 128],
                                  wqv6[:, :, o:o + 128])
            nc.sync.dma_start(x8v[:, :, 0:T], x86[:, :, 0:T])
            # fp8 k weights (needed by the k chains of group 0)
            wk8 = wres.tile([128, 6 * C], FP8, tag="wk8", name="wk8")
            wk6 = wk8.rearrange("p (c n) -> p c n", n=C)
            for o in range(0, C, 384):
                nc.sync.dma_start(wk6[:, :, o:o + 384],
                                  wk86[:, :, o:o + 384])
            # v weight columns (C stage)
            for o in range(C, 2 * C, 384):
                nc.sync.dma_start(wq6[:, :, o:o + 384],
                                  wqv6[:, :, o:o + 384])
            w_proj = wres.tile([128, 6 * C], BF16, tag="wproj", name="wproj")
            wp6 = w_proj.rearrange("p (c n) -> p c n", n=C)
            for o in range(0, C, 384):
                nc.sync.dma_start(wp6[:, :, o:o + 384],
                                  wproj6[:, :, o:o + 384])
            onesbd = wres.tile([128, 2], BF16, tag="onesbd")
            nc.sync.dma_start(onesbd[:], d_onesbd[:])
            bias_sb = wres.tile([128, 6], F32, tag="bias_sb")
            nc.sync.dma_start(bias_sb[:], d_bias[:])
            # exp tiles: dead bands (rows 49:64, 113:128) must stay zero.
            # 4 buffers: the tail pipeline is 2 steps deep, and each step
            # holds a pair (even j, odd j), so a tile written at step s is
            # still read at s+1 and can only be reused at s+2.
            eTs = []
            for nm in ("eTa0", "eTb0", "eTa1", "eTb1"):
                t = wres.tile([128, T], BF16, tag=nm, name=nm)
                nc.vector.memset(t[:], 0.0)
                eTs.append(t)
            # normalized exp tiles (only rows 0:49 / 64:113 ever read)
            eTns = []
            for nm in ("eTnA", "eTnB"):
                t = wres.tile([128, T], BF16, tag=nm, name=nm)
                eTns.append(t)
            # persistent double-buffered reciprocal staging tiles: the
            # 32-lane shuffle reads all 32 partitions, so they must be
            # fully initialized once (rows 2:32 stay at the memset value)
            rrbs, rros = [], []
            for nm in ("rrbA", "rrbB"):
                t = wres.tile([32, T], BF16, tag=nm, name=nm)
                nc.vector.memset(t[:], 0.0)
                rrbs.append(t)
            for nm in ("rroA", "rroB"):
                t = wres.tile([32, T], BF16, tag=nm, name=nm)
                nc.vector.memset(t[:], 0.0)
                rros.append(t)

            # ---- emission helpers ----
            def emit_B_chain(g, jc):
                st = S[g]
                if jc == 0:
                    st["q"], st["k"] = [None] * 6, [None] * 6
                pq = psA.tile([128, T], F32, tag="psA", name="psA")
                if jc < 6:
                    for ci in range(6):
                        nc.tensor.matmul(
                            pq[:], wq6[:, ci, 128 * jc:128 * (jc + 1)],
                            xv[:, ci, g * T:(g + 1) * T],
                            start=(ci == 0), stop=(ci == 5))
                else:
                    # fp8 DoubleRow: contract two c-chunks per matmul at
                    # 2x rate. 196-col halves keep the moving AP free dim
                    # at 392 <= 512 (the fast path). Both halves form ONE
                    # accumulation group: only the very first matmul sets
                    # start, so the bank's pending-zero covers the second
                    # half's columns instead of being re-marked (which
                    # would wipe the first half's results).
                    jk = jc - 6
                    for hh in range(2):
                        ts0 = g * T + 196 * hh
                        for c3 in range(3):
                            nc.tensor.matmul(
                                pq[:, 196 * hh:196 * (hh + 1)],
                                wk6[:, 2 * c3:2 * c3 + 2,
                                    128 * jk:128 * (jk + 1)],
                                x8v[:, 2 * c3:2 * c3 + 2, ts0:ts0 + 196],
                                start=(hh == 0 and c3 == 0),
                                stop=(hh == 1 and c3 == 2),
                                perf_mode=DR)
                nm = f"q{jc}" if jc < 6 else f"k{jc - 6}"
                t = p_qk.tile([128, T], BF16, tag=nm, name=nm)
                # all chain copies on ACT: GPSIMD cannot access PSUM
                # (BIR rule), and a DVE copy queues behind the deep
                # recip/shuffle/normalize chain, gating psA rotation
                nc.scalar.copy(t[:], pq[:])
                if jc < 6:
                    st["q"][jc] = t
                else:
                    st["k"][jc - 6] = t

            c_next = [0]

            def emit_C_unit():
                t4 = c_next[0]
                c_next[0] += 1
                tok0 = 128 * t4
                # scr layout [p, (parity, j, c)]: even-head features in
                # cols 0:384, odd in 384:768, so each scatter side is one
                # contiguous 384-element run per partition (cheap DMA
                # descriptor generation on the Sync engine)
                # host permuted the W_v columns to (parity, pair, c) order,
                # so half 0 of the GEMM output is all even-head features
                # and half 1 all odd-head: the copy is contiguous and each
                # scatter side is one 384-element run per partition
                scr = p_scr.tile([128, C], BF16, tag="scr", name="scr")
                for half in range(2):
                    pv = psS.tile([128, 384], F32, tag="psS", name="psS")
                    for ci in range(6):
                        nc.tensor.matmul(
                            pv[:], xv[:, ci, tok0:tok0 + 128],
                            wq6[:, ci, 768 + 384 * half:
                                768 + 384 * (half + 1)],
                            start=(ci == 0), stop=(ci == 5))
                    nc.vector.tensor_copy(
                        scr[:, 384 * half:384 * (half + 1)], pv[:])
                for b in range(tok0 // SEQ, min(B_CORE, (tok0 + 127) // SEQ + 1)):
                    lo = max(SEQ * b, tok0)
                    hi = min(SEQ * (b + 1), tok0 + 128)
                    if lo >= hi:
                        continue
                    gb, bb = b // G, b % G
                    if "v4" not in S[gb]:
                        vbd2 = p_vbd2.tile([128, G * 6 * 64], BF16,
                                           tag="vbd2", name="vbd2")
                        # [p, b, j, c]: per-(batch) 384-element contiguous
                        S[gb]["v4"] = vbd2.rearrange(
                            "p (b j c) -> p b j c", b=G, c=64)
                        S[gb]["v4f"] = vbd2.rearrange(
                            "p (b n) -> p b n", b=G)
                    v4f = S[gb]["v4f"]
                    sl, sh = lo - SEQ * b, hi - SEQ * b
                    nc.sync.dma_start(v4f[sl:sh, bb, :],
                                      scr[lo - tok0:hi - tok0, 0:384])
                    nc.sync.dma_start(v4f[64 + sl:64 + sh, bb, :],
                                      scr[lo - tok0:hi - tok0, 384:768])

            def ensure_C(tok_thresh):
                while c_next[0] < TOK // 128 and 128 * c_next[0] < tok_thresh:
                    emit_C_unit()

            def d_head(g, j):
                st = S[g]
                if j == 0:
                    st["unT"] = [p_unT.tile([128, T], BF16, tag=f"unT{ci}",
                                            name=f"unT{ci}")
                                 for ci in range(6)]
                    st["stash"] = {}
                q, k = st["q"], st["k"]
                eT = eTs[2 * (step_idx[0] % 2) + (j % 2)]
                ps = psS.tile([128, T], F32, tag="psS", name="psS")
                if g == 0 and j < 2:
                    # first-ever uses of the psS slots: make the dead band
                    # finite so the single exp below never sees raw psum
                    nc.vector.memset(ps[32:64, :], 0.0)
                for b in range(G):
                    bs = slice(49 * b, 49 * b + 49)
                    nc.tensor.matmul(ps[0:49, bs], k[j][0:64, bs],
                                     q[j][0:64, bs], start=True, stop=True)
                    nc.tensor.matmul(ps[64:113, bs], k[j][64:128, bs],
                                     q[j][64:128, bs], start=True, stop=True,
                                     tile_position=(64, 64))
                # ONE exp op: ACT cost scales with free size only, so a
                # single [0:113] pass costs half of two band passes. Rows
                # 49:64 hold stale-but-finite psum; their exps are killed
                # by onesbd zeros in the row-sum and never read via eTn.
                # (CoreSim reports NaN here — its fresh-tile memory has no
                # stale data — so this path is hardware-validated only.)
                nc.scalar.activation(eT[0:113, :], ps[0:113, :], EXP,
                                     scale=0.125)
                st["stash"][j] = [eT]

            def d_tail_a1(g, j):
                st = S[g]
                eT, = st["stash"][j]
                # per-pair base-0 psum tile: DVE lanes cannot shift
                # partitions, so the reciprocal must read rows 0:2
                pv_sum = psV.tile([2, T], F32, tag="psV", name="psV")
                nc.tensor.matmul(pv_sum[0:2, :], onesbd[:], eT[:],
                                 start=True, stop=True)
                st["stash"][j] = [eT, pv_sum]

            def d_tail_a2(g, j):
                st = S[g]
                eT, pv_sum = st["stash"][j]
                rr = p_rr.tile([2, T], F32, tag="rr", name="rr")
                nc.vector.reciprocal_approx_fast(rr[:], pv_sum[0:2, :])
                rrb = rrbs[j % 2]
                nc.vector.tensor_copy(rrb[0:2, :], rr[:])
                # partition_broadcast only reads partition 0, so move the
                # odd-head reciprocal (partition 1) to partition 0 of a
                # second tile via the DVE 32-lane shuffle
                rro = rros[j % 2]
                nc.vector.stream_shuffle(rro[:, :], rrb[:, :],
                                         mask=[1] + list(range(1, 32)))
                # per-token reciprocal broadcast across partitions on the
                # GPSIMD engine; output base partition must be 0 on HW, so
                # broadcast each head's reciprocal to all 128 partitions
                # and let the multiply read the matching half
                rbe = p_rb.tile([128, T], BF16, tag="rbE", name="rbE")
                nc.gpsimd.partition_broadcast(rbe[:, :], rrb[0:1, :],
                                              channels=128)
                rbo = p_rb.tile([128, T], BF16, tag="rbO", name="rbO")
                nc.gpsimd.partition_broadcast(rbo[:, :], rro[0:1, :],
                                              channels=128)
                # normalize the exp tile up front (a full double-step
                # before attn@V consumes it), so neither attn@V nor the
                # out-projection ever waits on the DVE/GPSIMD chain
                eTn = eTns[j % 2]
                nc.vector.tensor_mul(out=eTn[0:49, :], in0=eT[0:49, :],
                                     in1=rbe[0:49, :])
                nc.vector.tensor_mul(out=eTn[64:113, :], in0=eT[64:113, :],
                                     in1=rbo[64:113, :])
                st["stash"][j] = [eTn]

            def d_tail_b(g, j):
                st = S[g]
                eTn, = st["stash"][j]
                v4 = st["v4"]
                po = psO.tile([128, T], F32, tag="psO", name="psO")
                for b in range(G):
                    bs = slice(49 * b, 49 * b + 49)
                    nc.tensor.matmul(po[0:64, bs], v4[0:49, b, j, :],
                                     eTn[0:49, bs], start=True, stop=True)
                    nc.tensor.matmul(po[64:128, bs], v4[64:113, b, j, :],
                                     eTn[64:113, bs], start=True, stop=True,
                                     tile_position=(64, 64))
                st["stash"][j] = [po]

            def d_tail_c1(g, j):
                # ACT copy releases the po psum slot early (before E needs it)
                st = S[g]
                po, = st["stash"].pop(j)
                unT = st["unT"]
                nc.scalar.copy(unT[j][:], po[:])

            def emit_E(g, j2):
                st = S[g]
                unT = st["unT"]
                pp = psO.tile([128, T], F32, tag="psO", name="psO")
                for ci in range(6):
                    nc.tensor.matmul(
                        pp[:], wp6[:, ci, 128 * j2:128 * (j2 + 1)],
                        unT[ci][:], start=(ci == 0), stop=(ci == 5))
                osb = p_osb.tile([128, T], BF16, tag="osb", name="osb")
                # bias-add on ACT: it is the last reader of the pp psum
                # slot, and the ACT queue drains early each step, so psO
                # recycles in time for the next step's attn@V (a DVE
                # bias-add measured +53us: it gated psO from the DVE tail)
                nc.scalar.add(osb[:], pp[:], bias_sb[:, j2:j2 + 1])
                nc.sync.dma_start(out6[:, j2, g * T:(g + 1) * T],
                                  osb[:])

            step_idx = [0]

            # ---- bootstrap: group 0's B and C run standalone ----
            for jc in range(12):
                emit_B_chain(0, jc)
            load_xT(1)
            ensure_C(T)

            # ---- steady state: 3-stage pair pipeline. Step s emits, in
            # PE-queue order: attn@V of pair s-2 (consumes the normalized
            # exps produced a full step earlier, so it never waits on the
            # DVE/GPSIMD chain), the row-sums of pair s-1, the scores of
            # the current pair, then E units / next group's B and C. The
            # unT copies of pair s-2 go first on the ACT queue so the E
            # matmuls are never gated by them. The reciprocal + broadcast
            # + normalize chain of pair s-1 runs on DVE/GPSIMD with a
            # whole step of slack before attn@V reads its output. ----
            prev1 = None          # pair awaiting row-sum + normalize
            prev2 = None          # pair awaiting attn@V + unT copy
            e_queue = []          # (g, j2) E units awaiting emission

            def stage2(pg, p0, p1):
                d_tail_b(pg, p0)
                d_tail_b(pg, p1)
                d_tail_c1(pg, p0)
                d_tail_c1(pg, p1)
                if p1 == 5:
                    e_queue.extend((pg, j2) for j2 in range(6))

            for g in range(N_GROUPS):
                if g + 2 < N_GROUPS:
                    load_xT(g + 2)
                for jp in range(3):
                    if prev2:
                        stage2(*prev2)
                    if prev1:
                        d_tail_a1(prev1[0], prev1[1])
                        d_tail_a1(prev1[0], prev1[2])
                    d_head(g, 2 * jp)
                    d_head(g, 2 * jp + 1)
                    if prev1:
                        d_tail_a2(prev1[0], prev1[1])
                        d_tail_a2(prev1[0], prev1[2])
                    prev2, prev1 = prev1, (g, 2 * jp, 2 * jp + 1)
                    for _ in range(2):
                        if e_queue:
                            emit_E(*e_queue.pop(0))
                    if g + 1 < N_GROUPS:
                        for c4 in range(4):
                            emit_B_chain(g + 1, 4 * jp + c4)
                        ensure_C((g + 1) * T + (jp + 1) * T // 3)
                    step_idx[0] += 1

            # drain the two in-flight pairs, then the remaining E units
            for _ in range(2):
                if prev2:
                    stage2(*prev2)
                if prev1:
                    d_tail_a1(prev1[0], prev1[1])
                    d_tail_a1(prev1[0], prev1[2])
                    d_tail_a2(prev1[0], prev1[1])
                    d_tail_a2(prev1[0], prev1[2])
                prev2, prev1 = prev1, None
                for _ in range(2):
                    if e_queue:
                        emit_E(*e_queue.pop(0))
                step_idx[0] += 1
            for e in e_queue:
                emit_E(*e)

    nc.compile()
    return nc


def _prep_inputs(x, W_qkv, W_proj, b_proj):
    x = np.asarray(x, dtype=np.float32)
    B, N, Cc = x.shape
    consts = _consts()
    wq = np.asarray(W_qkv, dtype=np.float32)
    # permute W_v columns from (head, c) to (parity, pair, c) so the
    # C-stage GEMM emits even-head features in cols 0:384 and odd-head
    # in 384:768 (contiguous copy + contiguous scatter on device)
    wv = wq[:, 2 * Cc:3 * Cc].reshape(Cc, 6, 2, 64)
    wv = np.ascontiguousarray(wv.transpose(0, 2, 1, 3)).reshape(Cc, Cc)
    def tile_w(a):
        # [C, N] (c outer, p inner rows) -> partition-major [128, 6*N]
        n = a.shape[1]
        return np.ascontiguousarray(
            a.reshape(6, 128, n).transpose(1, 0, 2).reshape(128, 6 * n))

    wqv = tile_w(
        np.concatenate([wq[:, 0:Cc], wv], axis=1).astype(BF))
    wk8 = tile_w(wq[:, Cc:2 * Cc].astype(F8))
    wproj = tile_w(np.asarray(W_proj, dtype=np.float32).astype(BF))
    bias = np.ascontiguousarray(
        np.asarray(b_proj, dtype=np.float32).reshape(6, 128).T)
    x_bf = x.astype(BF)
    x_f8 = x.astype(F8)
    in_maps = []
    def tile_pm(a):
        # [C, TOK] feature-major -> partition-major [128, 6*TOK] matching
        # the SBUF layout run-for-run (c outer, p inner, as in "(c p) t")
        return np.ascontiguousarray(
            a.reshape(6, 128, TOK).transpose(1, 0, 2).reshape(128, 6 * TOK))

    for i in range(NUM_CORES):
        xt = tile_pm(x_bf[i * B_CORE:(i + 1) * B_CORE].reshape(TOK, Cc).T)
        xt8 = tile_pm(x_f8[i * B_CORE:(i + 1) * B_CORE].reshape(TOK, Cc).T)
        m = {"x": xt, "x8": xt8, "wqv": wqv, "wk8": wk8, "wproj": wproj,
             "bias": bias}
        m.update(consts)
        in_maps.append(m)
    return in_maps


def _unshard(results):
    out = np.empty((NUM_CORES * B_CORE, SEQ, C), dtype=np.float32)
    for i in range(NUM_CORES):
        o = np.asarray(results[i]["out"]).astype(np.float32)  # [128, 6*TOK]
        o = o.reshape(128, 6, TOK).transpose(1, 0, 2).reshape(C, TOK)
        out[i * B_CORE:(i + 1) * B_CORE] = o.T.reshape(B_CORE, SEQ, C)
    return out


def kernel(x, W_qkv, W_proj, b_proj):
    from concourse.bass_utils import run_bass_kernel_spmd

    if "nc" not in _CACHE:
        _CACHE["nc"] = _build()
    nc = _CACHE["nc"]

    in_maps = _prep_inputs(x, W_qkv, W_proj, b_proj)
    res = run_bass_kernel_spmd(nc, in_maps, list(range(NUM_CORES)))
    return _unshard(res.results)
